# revision 11
# baseline (speedup 1.0000x reference)
"""Multi-head attention (B=2, S=2048, D=1024, H=16, d_k=64) on 8 TRN2 NeuronCores.

Sharding: head-parallel. Core c owns heads (2c, 2c+1) for both batch rows:
 - replicated inputs: qT/kT/vT = x.reshape(B*S, D).T  in bf16, [1024, 4096]
   (D on partitions so the TensorEngine contracts over D with no transposes)
 - per-core weights: Wq columns / Wo rows for its two heads (host pre-packs
   wq/wqv d-major so each is ONE contiguous DMA)
 - per-core output: partial = attn_out(own heads) @ Wo[own rows]  [4096, 1024] bf16
   The host sums the 8 partials (f32) and adds bo.  No cross-core comm.

Per-core dataflow (bf16 matmuls, f32 PSUM):
 1. b0 x loads are [128,1024] chunks ordered (q cols 0:1024), (k all),
    (q cols 1024:2048), (v all) so qh j0/j1 projection starts ~1.5us in and
    scores(sc0) start as soon as kh completes (~20us).  b1 stays [128,2048].
 2. qhT/khT [128, 2048] per batch = Wq_c.T @ xT (+bq).  vh [2048, 130]
    natural = vT.T @ Wqv_c; Wqv has zero-cols / bqv has 1.0-cols so each
    head gets a ones column -> attn@V also produces softmax denominators.
 3. scoresT[t,s] = khT.T @ qhT, both heads packed into disjoint PE
    row-groups (K=64).  exp(x/8) on ScalarE from PSUM, bf16 out.
 4. attn@V accumulated over t; row 64 = denominator.  Normalize: DVE
    fast-reciprocal on the two [1,512] denominator rows, ONE K=2 f32r
    matmul broadcasts both heads' reciprocals across partitions, then two
    DVE muls produce normalized oT bf16.
 5. partial[s, :] = outT.T @ Wo_c -> bf16 ob [128,1024] (two PSUM copies)
    -> ONE DRAM store per 128 rows.

Scheduling: ScalarE (exp, ~140us) and the TensorEngine (~190us busy) --
PE is the bottleneck, so emission keeps the PE queue dense: each s-chunk's
scores+exp loop is emitted first; its attn@V/normalize/out-proj are
deferred one s-chunk and re-emitted between later score iterations via a
two-lane work queue (lane A: DMA-gated projection work with
earliest-iteration thresholds; lane B: deferred attention work, also
min-iter gated so no PE instruction is emitted before its input DMA can
have landed -- the PE queue is in-order, a stalled instruction blocks it).
"""

import numpy as np
import ml_dtypes

B, S, D, H, DK = 2, 2048, 1024, 16, 64
NCORES = 8
HPC = H // NCORES          # heads per core = 2
BS = B * S                 # 4096
HD = HPC * DK              # 128 = per-core head dims

_cache = {}


def _build():
    import concourse.bass as bass
    import concourse.tile as tile
    from concourse import bacc, mybir

    f32 = mybir.dt.float32
    f32r = mybir.dt.float32r
    bf16 = mybir.dt.bfloat16
    Exp = mybir.ActivationFunctionType.Exp

    nc = bacc.Bacc("TRN2", target_bir_lowering=False, debug=False,
                   num_devices=NCORES)

    qT = nc.declare_dram_parameter("qT", [D, BS], bf16, isOutput=False)
    kT = nc.declare_dram_parameter("kT", [D, BS], bf16, isOutput=False)
    vT = nc.declare_dram_parameter("vT", [D, BS], bf16, isOutput=False)
    ND = D // 128            # 8 d-chunks
    wq = nc.declare_dram_parameter("wq", [128, ND * HD], bf16, isOutput=False)
    wqv = nc.declare_dram_parameter("wqv", [128, ND * 130], bf16, isOutput=False)
    bq = nc.declare_dram_parameter("bq", [1, HD], f32, isOutput=False)
    bqv = nc.declare_dram_parameter("bqv", [1, 130], f32, isOutput=False)
    wo = nc.declare_dram_parameter("wo", [HD, D], bf16, isOutput=False)
    out = nc.declare_dram_parameter("out", [BS, D], bf16, isOutput=True)

    NT = S // 128            # 16 t-chunks per batch
    NSC = S // 512           # 4 s-chunks per batch

    with tile.TileContext(nc) as tc:
        with (
            tc.tile_pool(name="const", bufs=1) as pc,
            tc.tile_pool(name="xg", bufs=24) as pin1,
            tc.tile_pool(name="xb", bufs=12) as pinB,
            tc.tile_pool(name="proj", bufs=2) as pproj,
            tc.tile_pool(name="vh", bufs=2) as pvh,
            tc.tile_pool(name="exp", bufs=18) as pexp,
            tc.tile_pool(name="outT", bufs=2) as poutT,
            tc.tile_pool(name="small", bufs=2) as psmall,
            tc.tile_pool(name="ob", bufs=3) as pob,
            tc.tile_pool(name="ps", bufs=2, space="PSUM") as pps,
        ):
            # ---- constants: 5 single DMAs on the gpsimd queue ----
            bq_row = pc.tile([1, HD], f32)
            nc.gpsimd.dma_start(bq_row[:], bq[:, :])
            bqv_row = pc.tile([1, 130], f32)
            nc.gpsimd.dma_start(bqv_row[:], bqv[:, :])
            wq_sb = pc.tile([128, ND * HD], bf16)
            nc.gpsimd.dma_start(wq_sb[:], wq[:, :])
            wqv_sb = pc.tile([128, ND * 130], bf16)
            nc.gpsimd.dma_start(wqv_sb[:], wqv[:, :])
            wo_sb = pc.tile([HD, D], bf16)
            nc.gpsimd.dma_start(wo_sb[:], wo[:, :])

            def wqd(d):
                return wq_sb[:, d * HD:(d + 1) * HD]

            def wqvd(d):
                return wqv_sb[:, d * 130:(d + 1) * 130]

            # ---- b0 x loads: [128,1024] chunks, sync queue ----
            # order: q cols 0:1024 | k all | q cols 1024:2048 | v all
            def dma_grp(src, b, g, n):
                tiles = []
                for d in range(ND):
                    t = pin1.tile([128, 1024], bf16, tag="xg",
                                  name=f"x{n}{b}{g}{d}")
                    nc.sync.dma_start(
                        t[:], src[d * 128:(d + 1) * 128,
                                  b * S + g * 1024:b * S + (g + 1) * 1024])
                    tiles.append(t)
                return tiles

            qx = [None, None]
            kx = [None, None]
            vx = [None, None]
            qx[0] = dma_grp(qT, 0, 0, "q")
            kx[0] = dma_grp(kT, 0, 0, "k")
            kx[1] = dma_grp(kT, 0, 1, "k")
            qx[1] = dma_grp(qT, 0, 1, "q")
            vx[0] = dma_grp(vT, 0, 0, "v")
            vx[1] = dma_grp(vT, 0, 1, "v")

            ones_f = pc.tile([1, 128], f32)
            nc.vector.memset(ones_f[:], 1.0)
            ones_r = pc.tile([1, 128], f32r)
            nc.vector.tensor_copy(ones_r[:], ones_f[:])
            bq_row_r = pc.tile([1, HD], f32r)
            nc.vector.tensor_copy(bq_row_r[:], bq_row[:])
            bqv_row_r = pc.tile([1, 130], f32r)
            nc.vector.tensor_copy(bqv_row_r[:], bqv_row[:])
            # [1,128] bf16 head-selector rows for the denominator broadcast:
            # sel0 -> partitions 0:64, sel1 -> partitions 64:128
            self_f = pc.tile([1, 2, 128], f32)
            nc.vector.memset(self_f[:], 0.0)
            nc.vector.memset(self_f[0:1, 0, 0:64], 1.0)
            nc.vector.memset(self_f[0:1, 1, 64:128], 1.0)
            sel_b = pc.tile([1, 2, 128], bf16)
            nc.vector.tensor_copy(sel_b[:], self_f[:])
            sel0 = sel_b[0:1, 0, :]
            sel1 = sel_b[0:1, 1, :]

            # bq as per-partition column (qhT/khT bias) and broadcast
            # across partitions (vh bias, with the 1.0 ones-columns)
            ps_t = pps.tile([128, 128], f32, tag="p1")
            nc.tensor.matmul(ps_t, bq_row_r[:], ones_r[:],
                             start=True, stop=True)
            bq_col = pc.tile([128, 1], f32)
            nc.vector.tensor_copy(bq_col[:], ps_t[:, 0:1])
            ps_t2 = pps.tile([128, 130], f32, tag="p1")
            nc.tensor.matmul(ps_t2, ones_r[:], bqv_row_r[:],
                             start=True, stop=True)
            bqv_bc = pc.tile([128, 130], f32)
            nc.vector.tensor_copy(bqv_bc[:], ps_t2[:])

            # ---- b0 projections: qh j0/j1 chase the q DMA, then kh ----
            qh0 = pproj.tile([128, S], bf16, tag="projq", name="projq0")
            kh0 = pproj.tile([128, S], bf16, tag="projk", name="projk0")

            pq = [pps.tile([128, 512], f32, tag="p1", name=f"pq{j}")
                  for j in range(2)]
            for d in range(ND):
                for j in range(2):
                    nc.tensor.matmul(pq[j], wqd(d),
                                     qx[0][d][:, j * 512:(j + 1) * 512],
                                     start=(d == 0), stop=(d == ND - 1))
            for j in range(2):
                nc.vector.tensor_scalar_add(qh0[:, j * 512:(j + 1) * 512],
                                            pq[j], bq_col[:])
            for g in range(2):
                psk = pps.tile([128, 2, 512], f32, tag="sc", name=f"pk{g}")
                for d in range(ND):
                    for j in range(2):
                        nc.tensor.matmul(psk[:, j, :], wqd(d),
                                         kx[g][d][:, j * 512:(j + 1) * 512],
                                         start=(d == 0), stop=(d == ND - 1))
                for j in range(2):
                    jj = 2 * g + j
                    nc.vector.tensor_scalar_add(
                        kh0[:, jj * 512:(jj + 1) * 512], psk[:, j, :],
                        bq_col[:])

            # ---- two-lane deferred work queue ----
            laneA = []   # (min_iter, thunk): DMA-gated projection work
            laneB = []   # (min_iter, thunk): deferred attention work
            it = [0]

            def pump():
                popped = 0
                if laneA and laneA[0][0] <= it[0]:
                    laneA.pop(0)[1]()
                    popped = 1
                for _ in range(2 - popped):
                    if laneB and laneB[0][0] <= it[0]:
                        laneB.pop(0)[1]()
                it[0] += 1

            hold = {}

            # qh0 j2/j3 as lane-A items (q cols 1024:2048 land ~26us)
            def qj_thunk(j):
                def th():
                    ps = pps.tile([128, 512], f32, tag="p1", name=f"pqj{j}")
                    for d in range(ND):
                        nc.tensor.matmul(
                            ps, wqd(d),
                            qx[1][d][:, (j - 2) * 512:(j - 1) * 512],
                            start=(d == 0), stop=(d == ND - 1))
                    nc.vector.tensor_scalar_add(
                        qh0[:, j * 512:(j + 1) * 512], ps, bq_col[:])
                return th

            # vh items: b0 reads [128,1024] v chunks, b1 reads [128,2048]
            def vh_item(b, t, xv_of):
                def tt():
                    ps = pps.tile([128, 130], f32, tag="p1", name=f"pvh{b}{t}")
                    for d in range(ND):
                        nc.tensor.matmul(ps, xv_of(d, t), wqvd(d),
                                         start=(d == 0), stop=(d == ND - 1))
                    nc.vector.tensor_add(hold["vh" + str(b)][:, t, :],
                                         ps[:], bqv_bc[:])
                return tt

            def xv0_of(d, t):
                return vx[t // 8][d][:, (t % 8) * 128:(t % 8 + 1) * 128]

            def xv1_of(d, t):
                return hold["vt1"][d][:, t * 128:(t + 1) * 128]

            def dma_x8(src, b, n):
                tiles = []
                for d in range(ND):
                    t = pinB.tile([128, S], bf16, tag="xb", name=f"x{n}{b}{d}")
                    nc.sync.dma_start(t[:], src[d * 128:(d + 1) * 128,
                                                b * S:(b + 1) * S])
                    tiles.append(t)
                return tiles

            # b1 s-major projection as (min_iter, thunk) lane-A items
            def qk_chain_thunks(base):
                items = []

                def dma_thunk():
                    hold["qt1"] = dma_x8(qT, 1, "q")
                    hold["kt1"] = dma_x8(kT, 1, "k")
                    hold["qh"] = pproj.tile([128, S], bf16, tag="projq",
                                            name="projq1")
                    hold["kh"] = pproj.tile([128, S], bf16, tag="projk",
                                            name="projk1")
                items.append((0, dma_thunk))
                cell = {}
                for i, name in enumerate(("q", "k")):
                    for j in range(NSC):
                        def t1a(name=name, j=j):
                            ps = pps.tile([128, 512], f32,
                                          tag="p1", name=f"pb{name}{j}")
                            xt = hold["qt1" if name == "q" else "kt1"]
                            for d in range(4):
                                nc.tensor.matmul(ps, wqd(d),
                                                 xt[d][:, j * 512:(j + 1) * 512],
                                                 start=(d == 0), stop=False)
                            cell[(name, j)] = ps

                        def t1b(name=name, j=j):
                            ps = cell[(name, j)]
                            xt = hold["qt1" if name == "q" else "kt1"]
                            for d in range(4, ND):
                                nc.tensor.matmul(ps, wqd(d),
                                                 xt[d][:, j * 512:(j + 1) * 512],
                                                 start=False,
                                                 stop=(d == ND - 1))
                            sb = hold["qh" if name == "q" else "kh"]
                            nc.vector.tensor_scalar_add(
                                sb[:, j * 512:(j + 1) * 512], ps, bq_col[:])
                        items.append((base[i] + j, t1a))
                        items.append((base[i] + j, t1b))
                return items

            def norm_v2(att, oT, ssl, sfx):
                den = psmall.tile([1, 2, 512], f32, tag="den", name="den" + sfx)
                for h in range(HPC):
                    nc.vector.tensor_copy(den[0:1, h, :], att[h][64:65, :])
                rec = psmall.tile([1, 2, 512], f32, tag="rec",
                                  name="rec" + sfx)
                nc.vector.reciprocal_approx_fast(rec[:], den[:])
                recb = psmall.tile([1, 2, 512], bf16, tag="recb",
                                   name="recb" + sfx)
                nc.vector.tensor_copy(recb[:], rec[:])
                bcd = pps.tile([128, 512], f32, tag="p1", name="bcd" + sfx)
                nc.tensor.matmul(bcd, sel0, recb[0:1, 0, :],
                                 start=True, stop=False)
                nc.tensor.matmul(bcd, sel1, recb[0:1, 1, :],
                                 start=False, stop=True)
                bcs = psmall.tile([128, 512], f32, tag="bcs",
                                  name="bcs" + sfx)
                nc.vector.tensor_copy(bcs[:], bcd[:])
                for h in range(HPC):
                    nc.vector.tensor_mul(oT[h * 64:(h + 1) * 64, ssl],
                                         att[h][0:64, :],
                                         bcs[h * 64:(h + 1) * 64, :])

            def outproj_s1(b, sc, s1, oT, sfx):
                s0 = sc * 512 + s1 * 128
                rs = slice(b * S + s0, b * S + s0 + 128)
                ob = pob.tile([128, D], bf16, tag="ob", name="ob" + sfx)
                for n in range(2):
                    nsl = slice(n * 512, (n + 1) * 512)
                    ps = pps.tile([128, 512], f32, tag="p1",
                                  name="opps" + sfx)
                    nc.tensor.matmul(ps, oT[:, s0:s0 + 128], wo_sb[:, nsl],
                                     start=True, stop=True)
                    nc.vector.tensor_copy(ob[:, nsl], ps)
                nc.gpsimd.dma_start(out[rs, :], ob[:])

            def defer_attnv(b, sc, exs, vh_of, oT, gate):
                ssl = slice(sc * 512, (sc + 1) * 512)
                cell = {}
                for t in range(NT):
                    def av(t=t):
                        if t == 0:
                            cell["att"] = [
                                pps.tile([65, 512], f32, tag="att",
                                         name=f"att{b}{sc}{h}")
                                for h in range(HPC)]
                        vh = vh_of()
                        for h in range(HPC):
                            nc.tensor.matmul(cell["att"][h],
                                             vh[:, t, h * 65:h * 65 + 65],
                                             exs[t][:, h, :],
                                             start=(t == 0), stop=(t == NT - 1))
                    laneB.append((gate(t), av))

                def norm():
                    norm_v2(cell["att"], oT, ssl, f"{b}{sc}")
                laneB.append((0, norm))

                for g in range(2):
                    def op(g=g):
                        for u in range(2):
                            outproj_s1(b, sc, g * 2 + u, oT, f"{b}{sc}")
                    laneB.append((0, op))

            def inline_tail(b, sc, att, oT):
                norm_v2(att, oT, slice(sc * 512, (sc + 1) * 512), "L")
                for s1 in range(4):
                    outproj_s1(b, sc, s1, oT, "L")

            def attention(b, qh, kh, vh_of, gate0=None, last=False):
                oT = poutT.tile([128, S], bf16, tag="outT", name=f"oT{b}")
                for sc in range(NSC):
                    inline = last and sc == NSC - 1
                    exs = []
                    att = None
                    for t in range(NT):
                        pump()
                        scps = pps.tile([128, HPC, 512], f32, tag="sc")
                        for h in range(HPC):
                            hp = slice(h * 64, (h + 1) * 64)
                            nc.tensor.matmul(scps[:, h, :],
                                             kh[hp, t * 128:(t + 1) * 128],
                                             qh[hp, sc * 512:(sc + 1) * 512],
                                             start=True, stop=True)
                        ex = pexp.tile([128, HPC, 512], bf16, tag="exp")
                        nc.scalar.activation(ex[:], scps[:], Exp, scale=0.125)
                        exs.append(ex)
                        if inline:
                            if att is None:
                                att = [pps.tile([65, 512], f32, tag="att",
                                                name=f"attL{h}")
                                       for h in range(HPC)]
                            vh = vh_of()
                            for h in range(HPC):
                                nc.tensor.matmul(att[h],
                                                 vh[:, t, h * 65:h * 65 + 65],
                                                 ex[:, h, :],
                                                 start=(t == 0),
                                                 stop=(t == NT - 1))
                    if inline:
                        inline_tail(b, sc, att, oT)
                    else:
                        gate = gate0 if (gate0 is not None and sc == 0) \
                            else (lambda t: 0)
                        defer_attnv(b, sc, exs, vh_of, oT, gate)

            # ---- lane-A schedule ----
            # iters are scores-iterations (~1.1us each from ~20us).
            # landings: q1(cols 1024:) ~26us -> qh j2/j3 at 6/7;
            # v0 g0 ~32us -> vh t0..7 at 10+t; g1 ~38 -> t8.. at 17+;
            # b1: q1 ~50us -> 28+, k1 ~63us -> 40+, v1 ~75us -> 51+t
            hold["vh0"] = pvh.tile([128, NT, 130], bf16, tag="vh", name="vh0")
            items = [(6, qj_thunk(2)), (7, qj_thunk(3))]
            for t in range(NT):
                mi = 10 + t if t < 8 else 17 + (t - 8)
                items.append((mi, vh_item(0, t, xv0_of)))
            items += qk_chain_thunks((28, 40))

            def v1_thunk():
                hold["vt1"] = dma_x8(vT, 1, "v")
                hold["vh1"] = pvh.tile([128, NT, 130], bf16, tag="vh",
                                       name="vh1")
            items.append((20, v1_thunk))
            for t in range(NT):
                items.append((51 + t, vh_item(1, t, xv1_of)))
            laneA.extend(sorted(items, key=lambda x: x[0]))

            def gate_b0sc0(t):
                return 11 + t if t < 8 else 18 + (t - 8)

            attention(0, qh0, kh0, lambda: hold["vh0"], gate0=gate_b0sc0)
            attention(1, hold["qh"], hold["kh"], lambda: hold["vh1"],
                      last=True)
            wps = pps.tile([128, 512], f32, tag="p1", name="warmtail")

            def warm(n):
                for _ in range(n):
                    nc.tensor.matmul(wps, wqd(0), wq_sb[:, 0:512],
                                     start=True, stop=True,
                                     skip_group_check=True)

            while laneA or laneB:
                if laneA:
                    laneA.pop(0)[1]()
                if laneB:
                    laneB.pop(0)[1]()
                    if len(laneB) == 3:   # after last attnV, before norm
                        warm(8)
                    elif len(laneB) == 2:  # after norm, before outproj
                        warm(8)

    nc.compile()
    return nc


def make_in_maps(q, k, v, Wq, bq, Wo):
    bf = ml_dtypes.bfloat16
    ND = D // 128
    xT = {}
    for name, x in (("qT", q), ("kT", k), ("vT", v)):
        xT[name] = np.ascontiguousarray(
            np.asarray(x, np.float32).reshape(BS, D).T).astype(bf)

    in_maps = []
    for c in range(NCORES):
        cols = slice(c * HD, (c + 1) * HD)
        wqc = np.asarray(Wq, np.float32)[:, cols]
        bqc = np.asarray(bq, np.float32)[cols]
        wqve = np.zeros((D, 130), np.float32)
        wqve[:, 0:64] = wqc[:, 0:64]
        wqve[:, 65:129] = wqc[:, 64:128]
        bqve = np.zeros((1, 130), np.float32)
        bqve[0, 0:64] = bqc[0:64]
        bqve[0, 65:129] = bqc[64:128]
        bqve[0, 64] = 1.0
        bqve[0, 129] = 1.0
        # d-major packing: wq_pk[p, d*HD+c] = wqc[d*128+p, c]
        wq_pk = np.ascontiguousarray(
            wqc.reshape(ND, 128, HD).transpose(1, 0, 2).reshape(128, ND * HD))
        wqv_pk = np.ascontiguousarray(
            wqve.reshape(ND, 128, 130).transpose(1, 0, 2).reshape(128, ND * 130))
        in_maps.append({
            "qT": xT["qT"], "kT": xT["kT"], "vT": xT["vT"],
            "wq": wq_pk.astype(bf),
            "wqv": wqv_pk.astype(bf),
            "bq": bqc[None, :].copy(),
            "bqv": bqve,
            "wo": np.ascontiguousarray(np.asarray(Wo, np.float32)[cols, :]).astype(bf),
        })
    return in_maps


def kernel(q, k, v, Wq, bq, Wo, bo):
    import jax
    from concourse.bass_utils import run_bass_kernel_spmd

    try:
        jax.config.update("jax_compilation_cache_dir", "/tmp/jax_bass_cache")
        jax.config.update("jax_persistent_cache_min_entry_size_bytes", -1)
        jax.config.update("jax_persistent_cache_min_compile_time_secs", 0)
    except Exception:
        pass

    if "nc" not in _cache:
        _cache["nc"] = _build()
    nc = _cache["nc"]

    in_maps = make_in_maps(q, k, v, Wq, bq, Wo)
    res = run_bass_kernel_spmd(nc, in_maps, list(range(NCORES)), trace=False)
    acc = np.zeros((BS, D), np.float64)
    for c in range(NCORES):
        acc += res.results[c]["out"].astype(np.float64)
    acc += np.asarray(bo, np.float32)[None, :].astype(np.float64)
    return acc.reshape(B, S, D).astype(np.float32)


# revision 28
# speedup vs baseline: 1.0575x; 1.0575x over previous
"""Multi-head attention (B=2, S=2048, D=1024, H=16, d_k=64) on 8 TRN2 NeuronCores.

Sharding: head-parallel. Core c owns heads (2c, 2c+1) for both batch rows:
 - replicated inputs: qT/kT/vT = x.reshape(B*S, D).T  in bf16, [1024, 4096]
   (D on partitions so the TensorEngine contracts over D with no transposes)
 - per-core weights: Wq columns / Wo rows for its two heads (host pre-packs
   wq/wqv d-major so each is ONE contiguous DMA)
 - per-core output: partial = attn_out(own heads) @ Wo[own rows]  [4096, 1024] bf16
   The host sums the 8 partials (f32) and adds bo.  No cross-core comm.

Per-core dataflow (bf16 matmuls, f32 PSUM):
 1. b0 x loads are [128,1024] chunks ordered (q cols 0:1024), (k all),
    (q cols 1024:2048), (v all) so qh j0/j1 projection starts ~1.5us in and
    scores(sc0) start as soon as kh completes (~20us).  b1 stays [128,2048].
 2. qhT/khT [128, 2048] per batch = Wq_c.T @ xT (+bq).  vh [2048, 130]
    natural = vT.T @ Wqv_c; Wqv has zero-cols / bqv has 1.0-cols so each
    head gets a ones column -> attn@V also produces softmax denominators.
 3. scoresT[t,s] = khT.T @ qhT, both heads packed into disjoint PE
    row-groups (K=64).  exp(x/8) on ScalarE from PSUM, bf16 out.
 4. attn@V accumulated over t; row 64 = denominator.  Normalize: DVE
    fast-reciprocal on the two [1,512] denominator rows, ONE K=2 f32r
    matmul broadcasts both heads' reciprocals across partitions, then two
    DVE muls produce normalized oT bf16.
 5. partial[s, :] = outT.T @ Wo_c -> bf16 ob [128,1024] (two PSUM copies)
    -> ONE DRAM store per 128 rows.

Scheduling: ScalarE (exp, ~140us) and the TensorEngine (~190us busy) --
PE is the bottleneck, so emission keeps the PE queue dense: each s-chunk's
scores+exp loop is emitted first; its attn@V/normalize/out-proj are
deferred one s-chunk and re-emitted between later score iterations via a
two-lane work queue (lane A: DMA-gated projection work with
earliest-iteration thresholds; lane B: deferred attention work, also
min-iter gated so no PE instruction is emitted before its input DMA can
have landed -- the PE queue is in-order, a stalled instruction blocks it).
"""

import numpy as np
import ml_dtypes

B, S, D, H, DK = 2, 2048, 1024, 16, 64
NCORES = 8
HPC = H // NCORES          # heads per core = 2
BS = B * S                 # 4096
HD = HPC * DK              # 128 = per-core head dims

_cache = {}


def _build():
    import concourse.bass as bass
    import concourse.tile as tile
    from concourse import bacc, mybir

    f32 = mybir.dt.float32
    f32r = mybir.dt.float32r
    bf16 = mybir.dt.bfloat16
    Exp = mybir.ActivationFunctionType.Exp

    nc = bacc.Bacc("TRN2", target_bir_lowering=False, debug=False,
                   num_devices=NCORES)

    qT = nc.declare_dram_parameter("qT", [D, BS], bf16, isOutput=False)
    kT = nc.declare_dram_parameter("kT", [D, BS], bf16, isOutput=False)
    vT = nc.declare_dram_parameter("vT", [D, BS], bf16, isOutput=False)
    ND = D // 128            # 8 d-chunks
    wq = nc.declare_dram_parameter("wq", [128, ND * HD], bf16, isOutput=False)
    wqv = nc.declare_dram_parameter("wqv", [128, ND * 130], bf16, isOutput=False)
    bq = nc.declare_dram_parameter("bq", [1, HD], f32, isOutput=False)
    bqv = nc.declare_dram_parameter("bqv", [1, 130], f32, isOutput=False)
    wo = nc.declare_dram_parameter("wo", [HD, D], bf16, isOutput=False)
    out = nc.declare_dram_parameter("out", [BS, D], bf16, isOutput=True)

    NT = S // 128            # 16 t-chunks per batch
    NSC = S // 512           # 4 s-chunks per batch

    with tile.TileContext(nc) as tc:
        with (
            tc.tile_pool(name="const", bufs=1) as pc,
            tc.tile_pool(name="xg", bufs=24) as pin1,
            tc.tile_pool(name="xb", bufs=12) as pinB,
            tc.tile_pool(name="proj", bufs=2) as pproj,
            tc.tile_pool(name="vh", bufs=2) as pvh,
            tc.tile_pool(name="exp", bufs=19) as pexp,
            tc.tile_pool(name="outT", bufs=2) as poutT,
            tc.tile_pool(name="small", bufs=2) as psmall,
            tc.tile_pool(name="ob", bufs=3) as pob,
            tc.tile_pool(name="ps", bufs=2, space="PSUM") as pps,
        ):
            # ---- constants: 5 single DMAs on the gpsimd queue ----
            bq_row = pc.tile([1, HD], f32)
            nc.gpsimd.dma_start(bq_row[:], bq[:, :])
            bqv_row = pc.tile([1, 130], f32)
            nc.gpsimd.dma_start(bqv_row[:], bqv[:, :])
            wq_sb = pc.tile([128, ND * HD], bf16)
            nc.gpsimd.dma_start(wq_sb[:], wq[:, :])
            wqv_sb = pc.tile([128, ND * 130], bf16)
            nc.gpsimd.dma_start(wqv_sb[:], wqv[:, :])
            wo_sb = pc.tile([HD, D], bf16)
            nc.gpsimd.dma_start(wo_sb[:], wo[:, :])

            def wqd(d):
                return wq_sb[:, d * HD:(d + 1) * HD]

            def wqvd(d):
                return wqv_sb[:, d * 130:(d + 1) * 130]

            # ---- b0 x loads: [128,1024] chunks, sync queue ----
            # order: q cols 0:1024 | k all | q cols 1024:2048 | v all
            def dma_grp(src, b, g, n):
                tiles = []
                for d in range(ND):
                    t = pin1.tile([128, 1024], bf16, tag="xg",
                                  name=f"x{n}{b}{g}{d}")
                    nc.sync.dma_start(
                        t[:], src[d * 128:(d + 1) * 128,
                                  b * S + g * 1024:b * S + (g + 1) * 1024])
                    tiles.append(t)
                return tiles

            qx = [None, None]
            kx = [None, None]
            vx = [None, None]
            qx[0] = dma_grp(qT, 0, 0, "q")
            kx[0] = dma_grp(kT, 0, 0, "k")
            kx[1] = dma_grp(kT, 0, 1, "k")
            qx[1] = dma_grp(qT, 0, 1, "q")
            vx[0] = dma_grp(vT, 0, 0, "v")
            vx[1] = dma_grp(vT, 0, 1, "v")

            ones_f = pc.tile([1, 128], f32)
            nc.vector.memset(ones_f[:], 1.0)
            ones_r = pc.tile([1, 128], f32r)
            nc.vector.tensor_copy(ones_r[:], ones_f[:])
            bq_row_r = pc.tile([1, HD], f32r)
            nc.vector.tensor_copy(bq_row_r[:], bq_row[:])
            bqv_row_r = pc.tile([1, 130], f32r)
            nc.vector.tensor_copy(bqv_row_r[:], bqv_row[:])
            # [1,128] bf16 ones row: K=1 stationary for the denominator
            # broadcast matmuls (per-head, M=64)
            ones_b = pc.tile([1, 128], bf16)
            nc.vector.tensor_copy(ones_b[:], ones_f[:])

            # bq as per-partition column (qhT/khT bias) and broadcast
            # across partitions (vh bias, with the 1.0 ones-columns)
            ps_t = pps.tile([128, 128], f32, tag="p1")
            nc.tensor.matmul(ps_t, bq_row_r[:], ones_r[:],
                             start=True, stop=True)
            bq_col = pc.tile([128, 1], f32)
            nc.vector.tensor_copy(bq_col[:], ps_t[:, 0:1])
            ps_t2 = pps.tile([128, 130], f32, tag="p1")
            nc.tensor.matmul(ps_t2, ones_r[:], bqv_row_r[:],
                             start=True, stop=True)
            bqv_bc = pc.tile([128, 130], f32)
            nc.vector.tensor_copy(bqv_bc[:], ps_t2[:])

            # ---- b0 projections: qh j0/j1 chase the q DMA, then kh ----
            qh0 = pproj.tile([128, S], bf16, tag="projq", name="projq0")
            kh0 = pproj.tile([128, S], bf16, tag="projk", name="projk0")

            pq = [pps.tile([128, 512], f32, tag="p1", name=f"pq{j}")
                  for j in range(2)]
            for d in range(ND):
                for j in range(2):
                    nc.tensor.matmul(pq[j], wqd(d),
                                     qx[0][d][:, j * 512:(j + 1) * 512],
                                     start=(d == 0), stop=(d == ND - 1))
            for j in range(2):
                nc.vector.tensor_scalar_add(qh0[:, j * 512:(j + 1) * 512],
                                            pq[j], bq_col[:])
            for g in range(2):
                psk = pps.tile([128, 2, 512], f32, tag="sc", name=f"pk{g}")
                for d in range(ND):
                    for j in range(2):
                        nc.tensor.matmul(psk[:, j, :], wqd(d),
                                         kx[g][d][:, j * 512:(j + 1) * 512],
                                         start=(d == 0), stop=(d == ND - 1))
                for j in range(2):
                    jj = 2 * g + j
                    nc.vector.tensor_scalar_add(
                        kh0[:, jj * 512:(jj + 1) * 512], psk[:, j, :],
                        bq_col[:])

            # ---- three-lane deferred work queue ----
            laneA = []   # (min_iter, thunk): DMA-gated projection work
            laneB = []   # (min_iter, thunk): deferred attn@V + asb copies
            laneC = []   # (min_iter, ready_fn, thunk): norm + out-proj
            it = [0]

            def pump():
                popped = 0
                if laneA and laneA[0][0] <= it[0]:
                    laneA.pop(0)[1]()
                    popped = 1
                for _ in range(2 - popped):
                    if laneB and laneB[0][0] <= it[0]:
                        laneB.pop(0)[1]()
                if laneC and laneC[0][0] <= it[0] and laneC[0][1]():
                    laneC.pop(0)[2]()
                it[0] += 1

            hold = {}

            # qh0 j2/j3 as lane-A items (q cols 1024:2048 land ~26us)
            def qj_thunk(j):
                def th():
                    ps = pps.tile([128, 512], f32, tag="p1", name=f"pqj{j}")
                    for d in range(ND):
                        nc.tensor.matmul(
                            ps, wqd(d),
                            qx[1][d][:, (j - 2) * 512:(j - 1) * 512],
                            start=(d == 0), stop=(d == ND - 1))
                    nc.vector.tensor_scalar_add(
                        qh0[:, j * 512:(j + 1) * 512], ps, bq_col[:])
                return th

            # vh items: b0 reads [128,1024] v chunks, b1 reads [128,2048]
            def vh_item(b, t, xv_of):
                def tt():
                    ps = pps.tile([128, 130], f32, tag="p1", name=f"pvh{b}{t}")
                    for d in range(ND):
                        nc.tensor.matmul(ps, xv_of(d, t), wqvd(d),
                                         start=(d == 0), stop=(d == ND - 1))
                    nc.vector.tensor_add(hold["vh" + str(b)][:, t, :],
                                         ps[:], bqv_bc[:])
                return tt

            def xv0_of(d, t):
                return vx[t // 8][d][:, (t % 8) * 128:(t % 8 + 1) * 128]

            def xv1_of(d, t):
                return hold["vt1"][d][:, t * 128:(t + 1) * 128]

            def dma_x8(src, b, n):
                tiles = []
                for d in range(ND):
                    t = pinB.tile([128, S], bf16, tag="xb", name=f"x{n}{b}{d}")
                    nc.sync.dma_start(t[:], src[d * 128:(d + 1) * 128,
                                                b * S:(b + 1) * S])
                    tiles.append(t)
                return tiles

            # b1 s-major projection as (min_iter, thunk) lane-A items
            def qk_chain_thunks(base):
                items = []

                def dma_thunk():
                    hold["qt1"] = dma_x8(qT, 1, "q")
                    hold["kt1"] = dma_x8(kT, 1, "k")
                    hold["qh"] = pproj.tile([128, S], bf16, tag="projq",
                                            name="projq1")
                    hold["kh"] = pproj.tile([128, S], bf16, tag="projk",
                                            name="projk1")
                items.append((0, dma_thunk))
                cell = {}
                for i, name in enumerate(("q", "k")):
                    for j in range(NSC):
                        def t1a(name=name, j=j):
                            ps = pps.tile([128, 512], f32,
                                          tag="p1", name=f"pb{name}{j}")
                            xt = hold["qt1" if name == "q" else "kt1"]
                            for d in range(4):
                                nc.tensor.matmul(ps, wqd(d),
                                                 xt[d][:, j * 512:(j + 1) * 512],
                                                 start=(d == 0), stop=False)
                            cell[(name, j)] = ps

                        def t1b(name=name, j=j):
                            ps = cell[(name, j)]
                            xt = hold["qt1" if name == "q" else "kt1"]
                            for d in range(4, ND):
                                nc.tensor.matmul(ps, wqd(d),
                                                 xt[d][:, j * 512:(j + 1) * 512],
                                                 start=False,
                                                 stop=(d == ND - 1))
                            sb = hold["qh" if name == "q" else "kh"]
                            nc.vector.tensor_scalar_add(
                                sb[:, j * 512:(j + 1) * 512], ps, bq_col[:])
                        items.append((base[i] + j, t1a))
                        items.append((base[i] + j, t1b))
                return items

            def norm_v2(att, oT, ssl, sfx):
                # att may be the psum tiles (inline tail) or sbuf asb copies.
                # reciprocal on the den rows (base-64 aligned), bf16 rounding
                # copies feed two accumulating K=1 broadcast matmuls, then
                # per-head multiplies write normalized oT.
                den = psmall.tile([1, 2, 512], f32, tag="den",
                                  name="den" + sfx)
                for h in range(HPC):
                    nc.vector.tensor_copy(den[0:1, h, :], att[h][64:65, :])
                rec = psmall.tile([1, 2, 512], f32, tag="den",
                                  name="rec" + sfx)
                nc.vector.reciprocal_approx_fast(rec[:], den[:])
                recb = psmall.tile([1, 2, 512], bf16, tag="recb",
                                   name="recb" + sfx)
                nc.vector.tensor_copy(recb[:], rec[:])
                for h in range(HPC):
                    bcd = pps.tile([64, 512], f32, tag="p1",
                                   name=f"bcd{h}" + sfx)
                    nc.tensor.matmul(bcd, ones_b[0:1, 0:64],
                                     recb[0:1, h, :], start=True, stop=True)
                    bcs = psmall.tile([64, 512], f32, tag="bcs",
                                      name=f"bcs{h}" + sfx)
                    nc.vector.tensor_copy(bcs[:], bcd[:])
                    nc.vector.tensor_mul(oT[h * 64:(h + 1) * 64, ssl],
                                         att[h][0:64, :], bcs[:])

            def outproj_s1(b, sc, s1, oT, sfx, cast_eng=None):
                s0 = sc * 512 + s1 * 128
                rs = slice(b * S + s0, b * S + s0 + 128)
                ob = pob.tile([128, D], bf16, tag="ob", name="ob" + sfx)
                for n in range(2):
                    nsl = slice(n * 512, (n + 1) * 512)
                    ps = pps.tile([128, 512], f32, tag="p1",
                                  name="opps" + sfx)
                    nc.tensor.matmul(ps, oT[:, s0:s0 + 128], wo_sb[:, nsl],
                                     start=True, stop=True)
                    if cast_eng is None:
                        nc.vector.tensor_copy(ob[:, nsl], ps)
                    else:
                        cast_eng.copy(ob[:, nsl], ps)
                nc.gpsimd.dma_start(out[rs, :], ob[:])

            def defer_attnv(b, sc, exs, vh_of, oT, gate):
                ssl = slice(sc * 512, (sc + 1) * 512)
                cell = {}
                for t in range(NT):
                    def av(t=t):
                        if t == 0:
                            cell["att"] = [
                                pps.tile([65, 512], f32, tag="att",
                                         name=f"att{b}{sc}{h}")
                                for h in range(HPC)]
                        vh = vh_of()
                        for h in range(HPC):
                            nc.tensor.matmul(cell["att"][h],
                                             vh[:, t, h * 65:h * 65 + 65],
                                             exs[t][:, h, :],
                                             start=(t == 0), stop=(t == NT - 1))
                    laneB.append((gate(t), av))

                # asb copies (DVE-only) free the att psum bank promptly so
                # the next s-chunk's attn@V can start; the norm matmul +
                # out-proj go to laneC, gated past the last attn@V so score
                # matmuls sit between them in the in-order PE queue, hiding
                # the DVE reciprocal-chain latency.
                def asb_copy():
                    cell["asb"] = [
                        psmall.tile([65, 512], f32, tag="asb", bufs=4,
                                    name=f"asb{b}{sc}{h}")
                        for h in range(HPC)]
                    for h in range(HPC):
                        nc.vector.tensor_copy(cell["asb"][h][:],
                                              cell["att"][h][:])
                laneB.append((0, asb_copy))
                d0 = it[0]

                def norm():
                    norm_v2(cell["asb"], oT, ssl, f"{b}{sc}")
                    cell["normed"] = True
                laneC.append((d0 + 13, lambda: "asb" in cell, norm))

                for g in range(2):
                    def op(g=g):
                        for u in range(2):
                            outproj_s1(b, sc, g * 2 + u, oT, f"{b}{sc}")
                    laneC.append((d0 + 15 + g,
                                  lambda: "normed" in cell, op))

            def inline_tail(b, sc, att, oT):
                norm_v2(att, oT, slice(sc * 512, (sc + 1) * 512), "L")
                for s1 in range(4):
                    outproj_s1(b, sc, s1, oT, "L", cast_eng=nc.scalar)

            def attention(b, qh, kh, vh_of, gate0=None, last=False):
                oT = poutT.tile([128, S], bf16, tag="outT", name=f"oT{b}")
                for sc in range(NSC):
                    inline = last and sc == NSC - 1
                    exs = []
                    att = None
                    for t in range(NT):
                        pump()
                        scps = pps.tile([128, HPC, 512], f32, tag="sc")
                        for h in range(HPC):
                            hp = slice(h * 64, (h + 1) * 64)
                            nc.tensor.matmul(scps[:, h, :],
                                             kh[hp, t * 128:(t + 1) * 128],
                                             qh[hp, sc * 512:(sc + 1) * 512],
                                             start=True, stop=True)
                        ex = pexp.tile([128, HPC, 512], bf16, tag="exp")
                        nc.scalar.activation(ex[:], scps[:], Exp, scale=0.125)
                        exs.append(ex)
                        if inline:
                            if att is None:
                                att = [pps.tile([65, 512], f32, tag="att",
                                                name=f"attL{h}")
                                       for h in range(HPC)]
                            vh = vh_of()
                            for h in range(HPC):
                                nc.tensor.matmul(att[h],
                                                 vh[:, t, h * 65:h * 65 + 65],
                                                 ex[:, h, :],
                                                 start=(t == 0),
                                                 stop=(t == NT - 1))
                    if inline:
                        inline_tail(b, sc, att, oT)
                    else:
                        gate = gate0 if (gate0 is not None and sc == 0) \
                            else (lambda t: 0)
                        defer_attnv(b, sc, exs, vh_of, oT, gate)

            # ---- lane-A schedule ----
            # iters are scores-iterations (~1.1us each from ~20us).
            # landings: q1(cols 1024:) ~26us -> qh j2/j3 at 6/7;
            # v0 g0 ~32us -> vh t0..7 at 10+t; g1 ~38 -> t8.. at 17+;
            # b1: q1 ~50us -> 28+, k1 ~63us -> 40+, v1 ~75us -> 51+t
            hold["vh0"] = pvh.tile([128, NT, 130], bf16, tag="vh", name="vh0")
            items = [(6, qj_thunk(2)), (7, qj_thunk(3))]
            for t in range(NT):
                mi = 10 + t if t < 8 else 17 + (t - 8)
                items.append((mi, vh_item(0, t, xv0_of)))
            items += qk_chain_thunks((26, 38))

            def v1_thunk():
                hold["vt1"] = dma_x8(vT, 1, "v")
                hold["vh1"] = pvh.tile([128, NT, 130], bf16, tag="vh",
                                       name="vh1")
            items.append((20, v1_thunk))
            for t in range(NT):
                items.append((51 + t, vh_item(1, t, xv1_of)))
            laneA.extend(sorted(items, key=lambda x: x[0]))

            def gate_b0sc0(t):
                return 11 + t if t < 8 else 18 + (t - 8)

            attention(0, qh0, kh0, lambda: hold["vh0"], gate0=gate_b0sc0)
            attention(1, hold["qh"], hold["kh"], lambda: hold["vh1"],
                      last=True)
            wps = pps.tile([128, 512], f32, tag="p1", name="warmtail")

            def warm(n):
                for _ in range(n):
                    nc.tensor.matmul(wps, wqd(0), wq_sb[:, 0:512],
                                     start=True, stop=True,
                                     skip_group_check=True)

            while laneA or laneB or laneC:
                progressed = False
                if laneA:
                    laneA.pop(0)[1]()
                    progressed = True
                if laneB:
                    laneB.pop(0)[1]()
                    progressed = True
                if laneC and (laneC[0][1]() or not progressed):
                    laneC.pop(0)[2]()
                    if len(laneC) == 2:   # after norm, before outproj
                        warm(8)

    nc.compile()
    return nc


def make_in_maps(q, k, v, Wq, bq, Wo):
    bf = ml_dtypes.bfloat16
    ND = D // 128
    xT = {}
    for name, x in (("qT", q), ("kT", k), ("vT", v)):
        xT[name] = np.ascontiguousarray(
            np.asarray(x, np.float32).reshape(BS, D).T).astype(bf)

    in_maps = []
    for c in range(NCORES):
        cols = slice(c * HD, (c + 1) * HD)
        wqc = np.asarray(Wq, np.float32)[:, cols]
        bqc = np.asarray(bq, np.float32)[cols]
        wqve = np.zeros((D, 130), np.float32)
        wqve[:, 0:64] = wqc[:, 0:64]
        wqve[:, 65:129] = wqc[:, 64:128]
        bqve = np.zeros((1, 130), np.float32)
        bqve[0, 0:64] = bqc[0:64]
        bqve[0, 65:129] = bqc[64:128]
        bqve[0, 64] = 1.0
        bqve[0, 129] = 1.0
        # d-major packing: wq_pk[p, d*HD+c] = wqc[d*128+p, c]
        wq_pk = np.ascontiguousarray(
            wqc.reshape(ND, 128, HD).transpose(1, 0, 2).reshape(128, ND * HD))
        wqv_pk = np.ascontiguousarray(
            wqve.reshape(ND, 128, 130).transpose(1, 0, 2).reshape(128, ND * 130))
        in_maps.append({
            "qT": xT["qT"], "kT": xT["kT"], "vT": xT["vT"],
            "wq": wq_pk.astype(bf),
            "wqv": wqv_pk.astype(bf),
            "bq": bqc[None, :].copy(),
            "bqv": bqve,
            "wo": np.ascontiguousarray(np.asarray(Wo, np.float32)[cols, :]).astype(bf),
        })
    return in_maps


def kernel(q, k, v, Wq, bq, Wo, bo):
    import jax
    from concourse.bass_utils import run_bass_kernel_spmd

    try:
        jax.config.update("jax_compilation_cache_dir", "/tmp/jax_bass_cache")
        jax.config.update("jax_persistent_cache_min_entry_size_bytes", -1)
        jax.config.update("jax_persistent_cache_min_compile_time_secs", 0)
    except Exception:
        pass

    if "nc" not in _cache:
        _cache["nc"] = _build()
    nc = _cache["nc"]

    in_maps = make_in_maps(q, k, v, Wq, bq, Wo)
    res = run_bass_kernel_spmd(nc, in_maps, list(range(NCORES)), trace=False)
    acc = np.zeros((BS, D), np.float64)
    for c in range(NCORES):
        acc += res.results[c]["out"].astype(np.float64)
    acc += np.asarray(bo, np.float32)[None, :].astype(np.float64)
    return acc.reshape(B, S, D).astype(np.float32)


# revision 36
# speedup vs baseline: 1.0791x; 1.0204x over previous
"""Multi-head attention (B=2, S=2048, D=1024, H=16, d_k=64) on 8 TRN2 NeuronCores.

Sharding: head-parallel. Core c owns heads (2c, 2c+1) for both batch rows:
 - replicated inputs: qT/kT/vT = x.reshape(B*S, D).T  in bf16, [1024, 4096]
   (D on partitions so the TensorEngine contracts over D with no transposes)
 - per-core weights: Wq columns / Wo rows for its two heads (host pre-packs
   wq/wqv d-major so each is ONE contiguous DMA)
 - per-core output: partial = attn_out(own heads) @ Wo[own rows]  [4096, 1024] bf16
   The host sums the 8 partials (f32) and adds bo.  No cross-core comm.

Per-core dataflow (bf16 matmuls, f32 PSUM):
 1. b0 x loads are [128,1024] chunks ordered (q cols 0:1024), (k all),
    (q cols 1024:2048), (v all) so qh j0/j1 projection starts ~1.5us in and
    scores(sc0) start as soon as kh completes (~20us).  b1 stays [128,2048].
 2. qhT/khT [128, 2048] per batch = Wq_c.T @ xT (+bq).  vh [2048, 130]
    natural = vT.T @ Wqv_c; Wqv has zero-cols / bqv has 1.0-cols so each
    head gets a ones column -> attn@V also produces softmax denominators.
 3. scoresT[t,s] = khT.T @ qhT, both heads packed into disjoint PE
    row-groups (K=64).  exp(x/8) on ScalarE from PSUM, bf16 out.
 4. attn@V accumulated over t; row 64 = denominator.  Normalize: DVE
    fast-reciprocal on the two [1,512] denominator rows, ONE K=2 f32r
    matmul broadcasts both heads' reciprocals across partitions, then two
    DVE muls produce normalized oT bf16.
 5. partial[s, :] = outT.T @ Wo_c -> bf16 ob [128,1024] (two PSUM copies)
    -> ONE DRAM store per 128 rows.

Scheduling: ScalarE (exp, ~140us) and the TensorEngine (~190us busy) --
PE is the bottleneck, so emission keeps the PE queue dense: each s-chunk's
scores+exp loop is emitted first; its attn@V/normalize/out-proj are
deferred one s-chunk and re-emitted between later score iterations via a
two-lane work queue (lane A: DMA-gated projection work with
earliest-iteration thresholds; lane B: deferred attention work, also
min-iter gated so no PE instruction is emitted before its input DMA can
have landed -- the PE queue is in-order, a stalled instruction blocks it).
"""

import numpy as np
import ml_dtypes

B, S, D, H, DK = 2, 2048, 1024, 16, 64
NCORES = 8
HPC = H // NCORES          # heads per core = 2
BS = B * S                 # 4096
HD = HPC * DK              # 128 = per-core head dims

_cache = {}


def _build():
    import concourse.bass as bass
    import concourse.tile as tile
    from concourse import bacc, mybir

    f32 = mybir.dt.float32
    f32r = mybir.dt.float32r
    bf16 = mybir.dt.bfloat16
    Exp = mybir.ActivationFunctionType.Exp

    nc = bacc.Bacc("TRN2", target_bir_lowering=False, debug=False,
                   num_devices=NCORES)

    qT = nc.declare_dram_parameter("qT", [D, BS], bf16, isOutput=False)
    kT = nc.declare_dram_parameter("kT", [D, BS], bf16, isOutput=False)
    vT = nc.declare_dram_parameter("vT", [D, BS], bf16, isOutput=False)
    ND = D // 128            # 8 d-chunks
    wq = nc.declare_dram_parameter("wq", [128, ND * HD], bf16, isOutput=False)
    wqv = nc.declare_dram_parameter("wqv", [128, ND * 130], bf16, isOutput=False)
    bq = nc.declare_dram_parameter("bq", [1, HD], f32, isOutput=False)
    bqv = nc.declare_dram_parameter("bqv", [1, 130], f32, isOutput=False)
    wo = nc.declare_dram_parameter("wo", [HD, D], bf16, isOutput=False)
    out = nc.declare_dram_parameter("out", [BS, D], bf16, isOutput=True)

    NT = S // 128            # 16 t-chunks per batch
    NSC = S // 512           # 4 s-chunks per batch

    with tile.TileContext(nc) as tc:
        with (
            tc.tile_pool(name="const", bufs=1) as pc,
            tc.tile_pool(name="xg", bufs=24) as pin1,
            tc.tile_pool(name="proj", bufs=2) as pproj,
            tc.tile_pool(name="vh", bufs=2) as pvh,
            tc.tile_pool(name="exp", bufs=19) as pexp,
            tc.tile_pool(name="outT", bufs=2) as poutT,
            tc.tile_pool(name="small", bufs=2) as psmall,
            tc.tile_pool(name="ob", bufs=3) as pob,
            tc.tile_pool(name="ps", bufs=2, space="PSUM") as pps,
        ):
            # ---- constants: 5 single DMAs on the gpsimd queue ----
            bq_row = pc.tile([1, HD], f32)
            nc.gpsimd.dma_start(bq_row[:], bq[:, :])
            bqv_row = pc.tile([1, 130], f32)
            nc.gpsimd.dma_start(bqv_row[:], bqv[:, :])
            wq_sb = pc.tile([128, ND * HD], bf16)
            nc.gpsimd.dma_start(wq_sb[:], wq[:, :])
            wqv_sb = pc.tile([128, ND * 130], bf16)
            nc.gpsimd.dma_start(wqv_sb[:], wqv[:, :])
            wo_sb = pc.tile([HD, D], bf16)
            nc.gpsimd.dma_start(wo_sb[:], wo[:, :])

            def wqd(d):
                return wq_sb[:, d * HD:(d + 1) * HD]

            def wqvd(d):
                return wqv_sb[:, d * 130:(d + 1) * 130]

            # ---- x loads: [128,2048] d-chunks (4KB lines, full DMA rate),
            # sync queue.  b0 order: k | q | v so kh projection starts
            # ~2.5us in and scores(sc0) go as soon as qh j0 is done (~27us)
            def dma_x8(src, b, n):
                tiles = []
                for d in range(ND):
                    t = pin1.tile([128, S], bf16, tag="xg", name=f"x{n}{b}{d}")
                    nc.sync.dma_start(t[:], src[d * 128:(d + 1) * 128,
                                                b * S:(b + 1) * S])
                    tiles.append(t)
                return tiles

            kx0 = dma_x8(kT, 0, "k")
            qx0 = dma_x8(qT, 0, "q")
            vx0 = dma_x8(vT, 0, "v")

            ones_f = pc.tile([1, 128], f32)
            nc.vector.memset(ones_f[:], 1.0)
            ones_r = pc.tile([1, 128], f32r)
            nc.vector.tensor_copy(ones_r[:], ones_f[:])
            bq_row_r = pc.tile([1, HD], f32r)
            nc.vector.tensor_copy(bq_row_r[:], bq_row[:])
            bqv_row_r = pc.tile([1, 130], f32r)
            nc.vector.tensor_copy(bqv_row_r[:], bqv_row[:])
            # [1,128] bf16 ones row: K=1 stationary for the denominator
            # broadcast matmuls (per-head, M=64)
            ones_b = pc.tile([1, 128], bf16)
            nc.vector.tensor_copy(ones_b[:], ones_f[:])

            # ---- b0 projections: kh chases the k DMA, then qh ----
            qh0 = pproj.tile([128, S], bf16, tag="projq", name="projq0")
            kh0 = pproj.tile([128, S], bf16, tag="projk", name="projk0")

            bq_col = pc.tile([128, 1], f32)
            bqv_bc = pc.tile([128, 130], f32)

            def proj_mm(xt, nm):
                pss = [pps.tile([128, 2, 512], f32, tag="sc",
                                name=f"pj{nm}{i}") for i in range(2)]
                for d in range(ND):
                    for j in range(NSC):
                        nc.tensor.matmul(pss[j // 2][:, j % 2, :], wqd(d),
                                         xt[d][:, j * 512:(j + 1) * 512],
                                         start=(d == 0), stop=(d == ND - 1))
                return pss

            def proj_add(pss, sb):
                for j in range(NSC):
                    nc.vector.tensor_scalar_add(
                        sb[:, j * 512:(j + 1) * 512],
                        pss[j // 2][:, j % 2, :], bq_col[:])

            kh_ps = proj_mm(kx0, "k")
            # bias broadcasts sit AFTER the kh matmul stream on the PE
            # queue: their (coalesced) input waits are long satisfied by
            # the time PE reaches them, and the DVE bias-adds only need
            # bq_col at kh-projection end anyway.
            ps_t = pps.tile([128, 128], f32, tag="p1")
            nc.tensor.matmul(ps_t, bq_row_r[:], ones_r[:],
                             start=True, stop=True)
            nc.vector.tensor_copy(bq_col[:], ps_t[:, 0:1])
            ps_t2 = pps.tile([128, 130], f32, tag="p1")
            nc.tensor.matmul(ps_t2, ones_r[:], bqv_row_r[:],
                             start=True, stop=True)
            nc.vector.tensor_copy(bqv_bc[:], ps_t2[:])
            proj_add(kh_ps, kh0)
            qh_ps = proj_mm(qx0, "q")
            proj_add(qh_ps, qh0)

            # ---- three-lane deferred work queue ----
            laneA = []   # (min_iter, thunk): DMA-gated projection work
            laneB = []   # (min_iter, ready_fn, thunk): attn@V + asb copies
            laneC = []   # (min_iter, ready_fn, thunk): norm + out-proj
            it = [0]
            done = {}    # emission flags: (name, t) -> True

            def pump():
                popped = 0
                if laneA and laneA[0][0] <= it[0]:
                    laneA.pop(0)[1]()
                    popped = 1
                for _ in range(2 - popped):
                    if laneB and laneB[0][0] <= it[0] and laneB[0][1]():
                        laneB.pop(0)[2]()
                if laneC and laneC[0][0] <= it[0] and laneC[0][1]():
                    laneC.pop(0)[2]()
                it[0] += 1

            hold = {}

            # vh items: b0 reads [128,1024] v chunks, b1 reads [128,2048]
            def vh_item(b, t, xv_of):
                def tt():
                    ps = pps.tile([128, 130], f32, tag="p1", name=f"pvh{b}{t}")
                    for d in range(ND):
                        nc.tensor.matmul(ps, xv_of(d, t), wqvd(d),
                                         start=(d == 0), stop=(d == ND - 1))
                    nc.vector.tensor_add(hold["vh" + str(b)][:, t, :],
                                         ps[:], bqv_bc[:])
                    done[("vh" + str(b), t)] = True
                return tt

            def xv0_of(d, t):
                return vx0[d][:, t * 128:(t + 1) * 128]

            def xv1_of(d, t):
                return hold["vt1"][d][:, t * 128:(t + 1) * 128]

            # b1 s-major projection as (min_iter, thunk) lane-A items
            def qk_chain_thunks(base):
                items = []

                def dma_thunk():
                    hold["qt1"] = dma_x8(qT, 1, "q")
                    hold["kt1"] = dma_x8(kT, 1, "k")
                    hold["qh"] = pproj.tile([128, S], bf16, tag="projq",
                                            name="projq1")
                    hold["kh"] = pproj.tile([128, S], bf16, tag="projk",
                                            name="projk1")
                items.append((0, dma_thunk))
                cell = {}
                for i, name in enumerate(("q", "k")):
                    for j in range(NSC):
                        def t1a(name=name, j=j):
                            ps = pps.tile([128, 512], f32,
                                          tag="p1", name=f"pb{name}{j}")
                            xt = hold["qt1" if name == "q" else "kt1"]
                            for d in range(4):
                                nc.tensor.matmul(ps, wqd(d),
                                                 xt[d][:, j * 512:(j + 1) * 512],
                                                 start=(d == 0), stop=False)
                            cell[(name, j)] = ps

                        def t1b(name=name, j=j):
                            ps = cell[(name, j)]
                            xt = hold["qt1" if name == "q" else "kt1"]
                            for d in range(4, ND):
                                nc.tensor.matmul(ps, wqd(d),
                                                 xt[d][:, j * 512:(j + 1) * 512],
                                                 start=False,
                                                 stop=(d == ND - 1))
                            sb = hold["qh" if name == "q" else "kh"]
                            nc.vector.tensor_scalar_add(
                                sb[:, j * 512:(j + 1) * 512], ps, bq_col[:])
                        items.append((base[i] + j, t1a))
                        items.append((base[i] + j, t1b))
                return items

            def norm_v2(att, oT, ssl, sfx):
                # att may be the psum tiles (inline tail) or sbuf asb copies.
                # reciprocal on the den rows (base-64 aligned), bf16 rounding
                # copies feed two accumulating K=1 broadcast matmuls, then
                # per-head multiplies write normalized oT.
                den = psmall.tile([1, 2, 512], f32, tag="den",
                                  name="den" + sfx)
                for h in range(HPC):
                    nc.vector.tensor_copy(den[0:1, h, :], att[h][64:65, :])
                rec = psmall.tile([1, 2, 512], f32, tag="den",
                                  name="rec" + sfx)
                nc.vector.reciprocal_approx_fast(rec[:], den[:])
                recb = psmall.tile([1, 2, 512], bf16, tag="recb",
                                   name="recb" + sfx)
                nc.vector.tensor_copy(recb[:], rec[:])
                for h in range(HPC):
                    bcd = pps.tile([64, 512], f32, tag="p1",
                                   name=f"bcd{h}" + sfx)
                    nc.tensor.matmul(bcd, ones_b[0:1, 0:64],
                                     recb[0:1, h, :], start=True, stop=True)
                    bcs = psmall.tile([64, 512], f32, tag="bcs",
                                      name=f"bcs{h}" + sfx)
                    nc.vector.tensor_copy(bcs[:], bcd[:])
                    nc.vector.tensor_mul(oT[h * 64:(h + 1) * 64, ssl],
                                         att[h][0:64, :], bcs[:])

            def outproj_s1(b, sc, s1, oT, sfx, cast_eng=None):
                s0 = sc * 512 + s1 * 128
                rs = slice(b * S + s0, b * S + s0 + 128)
                ob = pob.tile([128, D], bf16, tag="ob", name="ob" + sfx)
                for n in range(2):
                    nsl = slice(n * 512, (n + 1) * 512)
                    ps = pps.tile([128, 512], f32, tag="p1",
                                  name="opps" + sfx)
                    nc.tensor.matmul(ps, oT[:, s0:s0 + 128], wo_sb[:, nsl],
                                     start=True, stop=True)
                    if cast_eng is None:
                        nc.vector.tensor_copy(ob[:, nsl], ps)
                    else:
                        cast_eng.copy(ob[:, nsl], ps)
                nc.gpsimd.dma_start(out[rs, :], ob[:])

            def defer_attnv(b, sc, exs, vh_of, oT, gate):
                ssl = slice(sc * 512, (sc + 1) * 512)
                cell = {}
                for t in range(NT):
                    def av(t=t):
                        if t == 0:
                            cell["att"] = [
                                pps.tile([65, 512], f32, tag="att",
                                         name=f"att{b}{sc}{h}")
                                for h in range(HPC)]
                        vh = vh_of()
                        for h in range(HPC):
                            nc.tensor.matmul(cell["att"][h],
                                             vh[:, t, h * 65:h * 65 + 65],
                                             exs[t][:, h, :],
                                             start=(t == 0), stop=(t == NT - 1))
                    laneB.append((gate(t),
                                  (lambda t=t: ("vh" + str(b), t) in done),
                                  av))

                # asb copies (DVE-only) free the att psum bank promptly so
                # the next s-chunk's attn@V can start; the norm matmul +
                # out-proj go to laneC, gated past the last attn@V so score
                # matmuls sit between them in the in-order PE queue, hiding
                # the DVE reciprocal-chain latency.
                def asb_copy():
                    cell["asb"] = [
                        psmall.tile([65, 512], f32, tag="asb", bufs=4,
                                    name=f"asb{b}{sc}{h}")
                        for h in range(HPC)]
                    for h in range(HPC):
                        nc.vector.tensor_copy(cell["asb"][h][:],
                                              cell["att"][h][:])
                laneB.append((0, lambda: True, asb_copy))
                d0 = it[0]

                def norm():
                    norm_v2(cell["asb"], oT, ssl, f"{b}{sc}")
                    cell["normed"] = True
                laneC.append((d0 + 13, lambda: "asb" in cell, norm))

                for g in range(2):
                    def op(g=g):
                        for u in range(2):
                            outproj_s1(b, sc, g * 2 + u, oT, f"{b}{sc}")
                    laneC.append((d0 + 15 + g,
                                  lambda: "normed" in cell, op))

            def inline_tail(b, sc, att, oT):
                norm_v2(att, oT, slice(sc * 512, (sc + 1) * 512), "L")
                for s1 in range(4):
                    outproj_s1(b, sc, s1, oT, "L", cast_eng=nc.scalar)

            def attention(b, qh, kh, vh_of, gate0=None, last=False):
                oT = poutT.tile([128, S], bf16, tag="outT", name=f"oT{b}")
                for sc in range(NSC):
                    inline = last and sc == NSC - 1
                    exs = []
                    att = None
                    for t in range(NT):
                        pump()
                        scps = pps.tile([128, HPC, 512], f32, tag="sc")
                        for h in range(HPC):
                            hp = slice(h * 64, (h + 1) * 64)
                            nc.tensor.matmul(scps[:, h, :],
                                             kh[hp, t * 128:(t + 1) * 128],
                                             qh[hp, sc * 512:(sc + 1) * 512],
                                             start=True, stop=True)
                        ex = pexp.tile([128, HPC, 512], bf16, tag="exp")
                        nc.scalar.activation(ex[:], scps[:], Exp, scale=0.125)
                        exs.append(ex)
                        if inline:
                            if att is None:
                                att = [pps.tile([65, 512], f32, tag="att",
                                                name=f"attL{h}")
                                       for h in range(HPC)]
                            vh = vh_of()
                            for h in range(HPC):
                                nc.tensor.matmul(att[h],
                                                 vh[:, t, h * 65:h * 65 + 65],
                                                 ex[:, h, :],
                                                 start=(t == 0),
                                                 stop=(t == NT - 1))
                    if inline:
                        inline_tail(b, sc, att, oT)
                    else:
                        gate = gate0 if (gate0 is not None and sc == 0) \
                            else (lambda t: 0)
                        defer_attnv(b, sc, exs, vh_of, oT, gate)

            # ---- lane-A schedule ----
            # iters are scores-iterations (~1.1us each from ~26.5us).
            # landings: v0 ~37.5us -> vh t at 10+t;
            # b1: q1 ~50us -> 21+, k1 ~62.5us -> 33+, v1 ~75us -> 45+t
            hold["vh0"] = pvh.tile([128, NT, 130], bf16, tag="vh", name="vh0")
            items = []
            for t in range(NT):
                items.append((10 + t, vh_item(0, t, xv0_of)))
            items += qk_chain_thunks((27, 38))

            def v1_thunk():
                hold["vt1"] = dma_x8(vT, 1, "v")
                hold["vh1"] = pvh.tile([128, NT, 130], bf16, tag="vh",
                                       name="vh1")
            items.append((20, v1_thunk))
            for t in range(NT):
                items.append((45 + t, vh_item(1, t, xv1_of)))
            laneA.extend(sorted(items, key=lambda x: x[0]))

            def gate_b0sc0(t):
                return 11 + t

            attention(0, qh0, kh0, lambda: hold["vh0"], gate0=gate_b0sc0)
            attention(1, hold["qh"], hold["kh"], lambda: hold["vh1"],
                      last=True)
            wps = pps.tile([128, 512], f32, tag="p1", name="warmtail")

            def warm(n):
                for _ in range(n):
                    nc.tensor.matmul(wps, wqd(0), wq_sb[:, 0:512],
                                     start=True, stop=True,
                                     skip_group_check=True)

            while laneA or laneB or laneC:
                progressed = False
                if laneA:
                    laneA.pop(0)[1]()
                    progressed = True
                if laneB and (laneB[0][1]() or not progressed):
                    laneB.pop(0)[2]()
                    progressed = True
                if laneC and (laneC[0][1]() or not progressed):
                    laneC.pop(0)[2]()
                    if len(laneC) == 2:   # after norm, before outproj
                        warm(8)

    nc.compile()
    return nc


def make_in_maps(q, k, v, Wq, bq, Wo):
    bf = ml_dtypes.bfloat16
    ND = D // 128
    xT = {}
    for name, x in (("qT", q), ("kT", k), ("vT", v)):
        xT[name] = np.ascontiguousarray(
            np.asarray(x, np.float32).reshape(BS, D).T).astype(bf)

    in_maps = []
    for c in range(NCORES):
        cols = slice(c * HD, (c + 1) * HD)
        wqc = np.asarray(Wq, np.float32)[:, cols]
        bqc = np.asarray(bq, np.float32)[cols]
        wqve = np.zeros((D, 130), np.float32)
        wqve[:, 0:64] = wqc[:, 0:64]
        wqve[:, 65:129] = wqc[:, 64:128]
        bqve = np.zeros((1, 130), np.float32)
        bqve[0, 0:64] = bqc[0:64]
        bqve[0, 65:129] = bqc[64:128]
        bqve[0, 64] = 1.0
        bqve[0, 129] = 1.0
        # d-major packing: wq_pk[p, d*HD+c] = wqc[d*128+p, c]
        wq_pk = np.ascontiguousarray(
            wqc.reshape(ND, 128, HD).transpose(1, 0, 2).reshape(128, ND * HD))
        wqv_pk = np.ascontiguousarray(
            wqve.reshape(ND, 128, 130).transpose(1, 0, 2).reshape(128, ND * 130))
        in_maps.append({
            "qT": xT["qT"], "kT": xT["kT"], "vT": xT["vT"],
            "wq": wq_pk.astype(bf),
            "wqv": wqv_pk.astype(bf),
            "bq": bqc[None, :].copy(),
            "bqv": bqve,
            "wo": np.ascontiguousarray(np.asarray(Wo, np.float32)[cols, :]).astype(bf),
        })
    return in_maps


def kernel(q, k, v, Wq, bq, Wo, bo):
    import jax
    from concourse.bass_utils import run_bass_kernel_spmd

    try:
        jax.config.update("jax_compilation_cache_dir", "/tmp/jax_bass_cache")
        jax.config.update("jax_persistent_cache_min_entry_size_bytes", -1)
        jax.config.update("jax_persistent_cache_min_compile_time_secs", 0)
    except Exception:
        pass

    if "nc" not in _cache:
        _cache["nc"] = _build()
    nc = _cache["nc"]

    in_maps = make_in_maps(q, k, v, Wq, bq, Wo)
    res = run_bass_kernel_spmd(nc, in_maps, list(range(NCORES)), trace=False)
    acc = np.zeros((BS, D), np.float64)
    for c in range(NCORES):
        acc += res.results[c]["out"].astype(np.float64)
    acc += np.asarray(bo, np.float32)[None, :].astype(np.float64)
    return acc.reshape(B, S, D).astype(np.float32)
